# revision 10
# baseline (speedup 1.0000x reference)
"""Trainium2 Bass kernel for the ASGRA GNN (3-layer GATv2 + pooling head).

Sharding: dst-node-range over 8 cores. Each core owns N/8 target nodes,
degree-sorts them, and processes its incoming edges in block-uniform
padded-degree grids of [128 nodes x D slots]. Per layer:
  - node transforms (lin_l/lin_r) are computed data-parallel on each core's
    own nodes (streamed per 128-node block: PE transpose + PE matmul),
  - xl is AllGathered so every core can gather xl[src] for arbitrary sources,
  - edge phase: dma_gather xl[src] and e_table[attr] into [128, dce, 256]
    tiles, fused leaky-relu + attention scores on DVE, exp on ACT,
    segment softmax = free-dim reduction over the padded-degree axis,
  - the padded-slot mask folds into exp via (s+30)*mask - 30,
  - weighted message sums accumulate unnormalized, divided by the softmax
    denominator once per block.
Graph mean-pool = indicator matmul + AllReduce; MLP head replicated.
"""

import numpy as np

P = 128
HID, H, C = 256, 4, 64
G = 64
NEG = 0.2
SHIFT = 30.0  # mask-fold shift: exp((s+SHIFT)*mask - SHIFT)
DEN_EPS = 1e-6

FULL_CFG = dict(N=30000, E=480000, NCORES=8, NUM_TOKENS=151, NUM_REL=51,
                EMB=256, BBOX=32, IN0=288, DCE=8)


def _derived(cfg):
    N, NCORES = cfg["N"], cfg["NCORES"]
    M = N // NCORES               # real nodes per core
    NB = (M + P - 1) // P         # node blocks per core
    MP = NB * P                   # padded nodes per core
    return M, NB, MP, NCORES * MP


def _wrap_idx(idx):
    """[S] int -> [128, S/16] int16 wrapped layout for dma_gather."""
    assert len(idx) % 16 == 0
    return np.ascontiguousarray(
        np.tile(idx.reshape(-1, 16).T, (8, 1)).astype(np.int16))


def preprocess(x, edge_index, edge_attr, batch, params, cfg):
    """Host-side sharding: per-core index/mask arrays + shared weights."""
    N, E, NCORES = cfg["N"], cfg["E"], cfg["NCORES"]
    M, NB, MP, NFULL = _derived(cfg)

    x = np.asarray(x, np.float32)
    ei = np.asarray(edge_index, np.int64)
    ea = np.asarray(edge_attr, np.int64)
    bt = np.asarray(batch, np.int64)
    src_g, dst_g = ei[0], ei[1]

    # ---- per-core degree-sorted node permutation + block schedule ----
    per_core = []
    deg_sorted_all = np.zeros((NCORES, MP), np.int64)
    for k in range(NCORES):
        lo = k * M
        emask = (dst_g >= lo) & (dst_g < lo + M)
        e_src = src_g[emask]
        e_dst = (dst_g[emask] - lo)
        e_att = ea[emask]
        deg = np.bincount(e_dst, minlength=M)
        perm = np.argsort(-deg, kind="stable")          # node order, degree desc
        deg_sorted_all[k, :M] = deg[perm]
        per_core.append(dict(lo=lo, perm=perm, e_src=e_src, e_dst=e_dst,
                             e_att=e_att, deg=deg))

    # shared block-degree schedule: max over cores so SPMD program is uniform
    Ds = [max(1, int(deg_sorted_all[:, b * P:(b + 1) * P].max()))
          for b in range(NB)]
    S_off = np.concatenate([[0], np.cumsum([P * D for D in Ds])]).astype(np.int64)
    S = int(S_off[-1])

    # ---- per-core slot arrays ----
    for k, cd in enumerate(per_core):
        perm, deg = cd["perm"], cd["deg"]
        starts = np.zeros(M + 1, np.int64)
        np.cumsum(deg, out=starts[1:])
        order = np.argsort(cd["e_dst"], kind="stable")
        gidx = np.zeros(S, np.int64)        # src gather index (permuted global)
        aidx = np.zeros(S, np.int64)        # e_table row
        mcol = np.zeros((P, S // P), np.float32)   # mask in [p, slot-col]

        # map original global node id -> permuted-global gather position
        # (only for this core's own nodes; done once globally below)
        for b in range(NB):
            D = Ds[b]
            base = S_off[b]
            for i in range(P):
                pos = b * P + i
                if pos >= M:
                    continue
                n = perm[pos]
                d = deg[n]
                eids = order[starts[n]:starts[n] + d]
                ss = cd["e_src"][eids]
                aa = cd["e_att"][eids]
                # slot (b, j, p) at flat index base + j*128 + p
                gidx[base + np.arange(d) * P + i] = ss   # placeholder: global id
                aidx[base + np.arange(d) * P + i] = aa
                mcol[i, base // P + np.arange(d)] = 1.0
        cd["gidx_raw"] = gidx
        cd["aidx"] = aidx
        cd["mcol"] = mcol

    # global -> permuted-global position map
    g2p = np.zeros(N, np.int64)
    for k, cd in enumerate(per_core):
        inv = np.argsort(cd["perm"])
        g2p[cd["lo"]:cd["lo"] + M] = k * MP + inv

    tok_full = x[:, 0].astype(np.int64)
    bbox_full = x[:, 1:5].astype(np.float32)

    cnt = np.bincount(bt, minlength=G).astype(np.float32)
    invcnt = (1.0 / np.maximum(cnt, 1.0)).astype(np.float32).reshape(G, 1)

    core_inputs = []
    for k, cd in enumerate(per_core):
        perm = cd["perm"]
        gidx = g2p[cd["gidx_raw"]]
        own_glob = np.zeros(MP, np.int64)
        own_glob[:M] = cd["lo"] + perm       # original ids in permuted order
        tok_idx = np.zeros(MP, np.int64)
        tok_idx[:M] = tok_full[own_glob[:M]]
        bbox_r = np.zeros((MP, 4), np.float32)
        bbox_r[:M] = bbox_full[own_glob[:M]]
        bbox_r = np.ascontiguousarray(
            bbox_r.reshape(-1, P, 4).transpose(1, 0, 2).reshape(P, -1))
        ind = np.zeros((P, NB * G), np.float32)
        for b in range(NB):
            for i in range(P):
                pos = b * P + i
                if pos < M:
                    ind[i, b * G + bt[own_glob[pos]]] = 1.0
        core_inputs.append(dict(
            gidx=_wrap_idx(gidx), aidx=_wrap_idx(cd["aidx"]),
            mask=np.ascontiguousarray(cd["mcol"]),
            tok_idx=_wrap_idx(tok_idx),
            bbox_r=bbox_r, ind=ind, invcnt=invcnt))

    # ---- shared weights, preshaped ----
    p = params
    def A(v):
        return np.ascontiguousarray(np.asarray(v, np.float32))
    shared = dict(
        tok_emb=A(p["tok_emb"]),
        bbW=A(p["bbox_proj"]["W"]),
        bbb=A(p["bbox_proj"]["b"]).reshape(1, -1),
        relT=A(np.asarray(p["rel_emb"], np.float32).T),   # [C, NUM_REL]
        W1=A(p["mlp1"]["W"]), b1=A(p["mlp1"]["b"]).reshape(1, -1),
        W2=A(p["mlp2"]["W"]), b2=A(p["mlp2"]["b"]).reshape(1, -1),
    )
    for li, conv in enumerate(p["convs"]):
        shared[f"Wl{li}"] = A(conv["lin_l"]["W"])
        shared[f"Wr{li}"] = A(conv["lin_r"]["W"])
        shared[f"bl{li}"] = A(conv["lin_l"]["b"]).reshape(1, -1)
        shared[f"br{li}"] = A(conv["lin_r"]["b"]).reshape(1, -1)
        shared[f"bias{li}"] = A(conv["bias"]).reshape(1, -1)
        shared[f"att{li}"] = A(np.asarray(conv["att"], np.float32).reshape(1, -1))
        shared[f"leW{li}"] = A(conv["lin_e"]["W"])
    return Ds, S_off, core_inputs, shared


# --------------------------------------------------------------------------
def build_program(cfg, Ds, S_off, num_cores):
    import concourse.bacc as bacc
    import concourse.mybir as mybir
    import concourse.tile as tile
    from concourse.masks import make_identity

    N = cfg["N"]
    NUM_TOKENS, NUM_REL = cfg["NUM_TOKENS"], cfg["NUM_REL"]
    EMB, BBOX, IN0, DCE = cfg["EMB"], cfg["BBOX"], cfg["IN0"], cfg["DCE"]
    M, NB, MP, NFULL = _derived(cfg)
    S = int(S_off[-1])
    f32 = mybir.dt.float32
    i16 = mybir.dt.int16
    AO = mybir.AluOpType
    AF = mybir.ActivationFunctionType
    RG = [list(range(num_cores))]

    nc = bacc.Bacc("TRN2", target_bir_lowering=False, debug=False,
                   num_devices=num_cores)

    # ---------------- I/O ----------------
    din = {}
    def inp(name, shape, dt=f32):
        din[name] = nc.dram_tensor(name, list(shape), dt, kind="ExternalInput")
        return din[name]

    gidx_d = inp("gidx", (P, S // 16), i16)
    aidx_d = inp("aidx", (P, S // 16), i16)
    mask_d = inp("mask", (P, S // P))
    tok_idx_d = inp("tok_idx", (P, MP // 16), i16)
    bbox_r_d = inp("bbox_r", (P, NB * 4))
    ind_d = inp("ind", (P, NB * G))
    invcnt_d = inp("invcnt", (G, 1))
    tok_emb_d = inp("tok_emb", (NUM_TOKENS, EMB))
    bbW_d = inp("bbW", (4, BBOX))
    bbb_d = inp("bbb", (1, BBOX))
    relT_d = inp("relT", (C, NUM_REL))
    W1_d = inp("W1", (HID, HID)); b1_d = inp("b1", (1, HID))
    W2_d = inp("W2", (HID, 8)); b2_d = inp("b2", (1, 8))
    Fs = [IN0, HID, HID]
    for li in range(3):
        inp(f"Wl{li}", (Fs[li], HID)); inp(f"Wr{li}", (Fs[li], HID))
        inp(f"bl{li}", (1, HID)); inp(f"br{li}", (1, HID))
        inp(f"bias{li}", (1, HID)); inp(f"att{li}", (1, HID))
        inp(f"leW{li}", (C, HID))
    out_d = nc.dram_tensor("out", [G, 8], f32, kind="ExternalOutput")

    with tile.TileContext(nc) as tc:
        with (
            tc.tile_pool(name="const", bufs=1) as cp,
            tc.tile_pool(name="stream", bufs=3) as sp,
            tc.tile_pool(name="edge", bufs=3) as ep,
            tc.tile_pool(name="acc", bufs=2) as ap_,
            tc.tile_pool(name="psT", bufs=2, space="PSUM") as psT,
            tc.tile_pool(name="psM", bufs=2, space="PSUM") as psM,
            tc.tile_pool(name="psS", bufs=1, space="PSUM") as psS,
            tc.tile_pool(name="dram", bufs=1, space="DRAM") as dp,
        ):
            ident = cp.tile([P, P], f32)
            make_identity(nc, ident[:])
            negS = cp.tile([P, 1], f32)
            nc.gpsimd.memset(negS[:], -SHIFT)

            # persistent SBUF loads
            gidx = cp.tile([P, S // 16], i16)
            nc.sync.dma_start(gidx[:], gidx_d[:])
            aidx = cp.tile([P, S // 16], i16)
            nc.sync.dma_start(aidx[:], aidx_d[:])
            maskc = cp.tile([P, S // P], f32)
            nc.sync.dma_start(maskc[:], mask_d[:])
            tok_idx = cp.tile([P, MP // 16], i16)
            nc.sync.dma_start(tok_idx[:], tok_idx_d[:])
            ind = cp.tile([P, NB * G], f32)
            nc.sync.dma_start(ind[:], ind_d[:])
            invcnt = cp.tile([G, 1], f32)
            nc.sync.dma_start(invcnt[:], invcnt_d[:])
            bbox_r = cp.tile([P, NB * 4], f32)
            nc.sync.dma_start(bbox_r[:], bbox_r_d[:])
            bbW = cp.tile([4, BBOX], f32)
            nc.sync.dma_start(bbW[:], bbW_d[:])
            bbb_rep = cp.tile([P, BBOX], f32)
            nc.sync.dma_start(bbb_rep[:], bbb_d[:].to_broadcast([P, BBOX]))
            relT = cp.tile([C, NUM_REL], f32)
            nc.sync.dma_start(relT[:], relT_d[:])

            WL, WR, BLr, BRr, BIASr, ATTr, LEW = [], [], [], [], [], [], []
            for li in range(3):
                F = Fs[li]
                KG = (F + P - 1) // P
                wl = cp.tile([P, KG, HID], f32, name=f"wl{li}", tag=f"wl{li}")
                wr = cp.tile([P, KG, HID], f32, name=f"wr{li}", tag=f"wr{li}")
                for g in range(KG):
                    ksz = min(P, F - g * P)
                    nc.sync.dma_start(wl[:ksz, g, :], din[f"Wl{li}"][g * P:g * P + ksz, :])
                    nc.sync.dma_start(wr[:ksz, g, :], din[f"Wr{li}"][g * P:g * P + ksz, :])
                WL.append(wl); WR.append(wr)
                for lst, nm in ((BLr, "bl"), (BRr, "br"), (BIASr, "bias"), (ATTr, "att")):
                    t = cp.tile([P, HID], f32, name=f"{nm}{li}r", tag=f"{nm}{li}r")
                    nc.sync.dma_start(t[:], din[f"{nm}{li}"][:].to_broadcast([P, HID]))
                    lst.append(t)
                lew = cp.tile([C, HID], f32, name=f"lew{li}", tag=f"lew{li}")
                nc.sync.dma_start(lew[:], din[f"leW{li}"][:])
                LEW.append(lew)

            W1 = cp.tile([P, 2, HID], f32)
            W2 = cp.tile([P, 2, 8], f32)
            for g in range(2):
                nc.sync.dma_start(W1[:, g, :], W1_d[g * P:(g + 1) * P, :])
                nc.sync.dma_start(W2[:, g, :], W2_d[g * P:(g + 1) * P, :])
            b1_rep = cp.tile([G, HID], f32)
            nc.sync.dma_start(b1_rep[:], b1_d[:].to_broadcast([G, HID]))
            b2_rep = cp.tile([G, 8], f32)
            nc.sync.dma_start(b2_rep[:], b2_d[:].to_broadcast([G, 8]))

            # node features of current layer, row-major [p, block, ch]
            h_row = cp.tile([P, NB, HID], f32)

            # per-layer DRAM buffers
            xl_own = [dp.tile([MP, HID], f32, name=f"xl_own{i}") for i in range(3)]
            xl_full = [dp.tile([NFULL, HID], f32, name=f"xl_full{i}") for i in range(3)]
            xr_own = [dp.tile([MP, HID], f32, name=f"xr_own{i}") for i in range(3)]
            e_tab = [dp.tile([NUM_REL, HID], f32, name=f"e_tab{i}") for i in range(3)]
            pool_in = dp.tile([G, HID], f32)
            pool_out = dp.tile([G, HID], f32)

            for li in range(3):
                F = Fs[li]
                KG = (F + P - 1) // P

                # ---- e_table = rel_emb @ lin_e.W ----
                et_ps = psM.tile([NUM_REL, HID], f32, space="PSUM", tag="mmps")
                nc.tensor.matmul(et_ps[:], lhsT=relT[:], rhs=LEW[li][:],
                                 start=True, stop=True)
                et_sb = sp.tile([NUM_REL, HID], f32, tag="etsb")
                nc.scalar.copy(et_sb[:], et_ps[:])
                nc.sync.dma_start(e_tab[li][:], et_sb[:])

                # ---- node transforms, streamed per block ----
                for b in range(NB):
                    if li == 0:
                        # layer-0 input built on the fly: tok emb gather + bbox
                        tok_blk = sp.tile([P, 1, EMB], f32, tag="tokb")
                        nc.gpsimd.dma_gather(
                            tok_blk[:], tok_emb_d[:],
                            tok_idx[:, b * 8:(b + 1) * 8], P, P, EMB)
                        bbT_ps = psT.tile([P, P], f32, space="PSUM", tag="tps")
                        nc.tensor.transpose(out=bbT_ps[:4, :],
                                            in_=bbox_r[:, b * 4:(b + 1) * 4],
                                            identity=ident[:])
                        bbT = sp.tile([4, P], f32, tag="bbT")
                        nc.scalar.copy(bbT[:], bbT_ps[:4, :])
                        bb_ps = psM.tile([P, BBOX], f32, space="PSUM", tag="mmps")
                        nc.tensor.matmul(bb_ps[:], lhsT=bbT[:],
                                         rhs=bbW[:], start=True, stop=True)
                        bb_sb = sp.tile([P, BBOX], f32, tag="bbsb")
                        nc.vector.tensor_add(bb_sb[:], bb_ps[:], bbb_rep[:])
                    hT = sp.tile([P, KG, P], f32, tag="hT")
                    for g in range(KG):
                        ksz = min(P, F - g * P)
                        if li == 0:
                            src_ap = (tok_blk[:, 0, g * P:g * P + ksz] if g < 2
                                      else bb_sb[:])
                        else:
                            src_ap = h_row[:, b, g * P:g * P + ksz]
                        tp = psT.tile([P, P], f32, space="PSUM", tag="tps")
                        nc.tensor.transpose(out=tp[:ksz, :], in_=src_ap,
                                            identity=ident[:])
                        nc.scalar.copy(hT[:ksz, g, :], tp[:ksz, :])
                    for dst_dram, W, brep in ((xl_own[li], WL[li], BLr[li]),
                                              (xr_own[li], WR[li], BRr[li])):
                        mm = psM.tile([P, HID], f32, space="PSUM", tag="mmps")
                        for g in range(KG):
                            ksz = min(P, F - g * P)
                            nc.tensor.matmul(mm[:], lhsT=hT[:ksz, g, :],
                                             rhs=W[:ksz, g, :],
                                             start=(g == 0), stop=(g == KG - 1))
                        xsb = sp.tile([P, HID], f32, tag="xsb")
                        nc.vector.tensor_add(xsb[:], mm[:], brep[:])
                        nc.sync.dma_start(dst_dram[b * P:(b + 1) * P, :], xsb[:])

                # ---- AllGather xl ----
                nc.gpsimd.collective_compute(
                    "AllGather", AO.bypass, replica_groups=RG,
                    ins=[xl_own[li].opt()], outs=[xl_full[li].opt()])

                # ---- edge phase ----
                for b in range(NB):
                    D = Ds[b]
                    base = int(S_off[b])
                    xr_blk = sp.tile([P, HID], f32, tag="xrb")
                    nc.sync.dma_start(xr_blk[:], xr_own[li][b * P:(b + 1) * P, :])
                    wmsg = ap_.tile([P, HID], f32, tag="wmsg")
                    den = ap_.tile([P, H], f32, tag="den")
                    for j0 in range(0, D, DCE):
                        dce = min(DCE, D - j0)
                        ni = dce * P
                        co = (base + j0 * P) // 16
                        mc = base // P + j0
                        xlg = ep.tile([P, DCE, HID], f32, tag="xlg")
                        nc.gpsimd.dma_gather(
                            xlg[:, :dce, :], xl_full[li][:],
                            gidx[:, co:co + ni // 16], ni, ni, HID)
                        eg = ep.tile([P, DCE, HID], f32, tag="eg")
                        nc.gpsimd.dma_gather(
                            eg[:, :dce, :], e_tab[li][:],
                            aidx[:, co:co + ni // 16], ni, ni, HID)
                        # m = xlg + eg + xr (into eg, then lrelu in place)
                        nc.vector.tensor_add(eg[:, :dce, :], xlg[:, :dce, :],
                                             eg[:, :dce, :])
                        nc.vector.tensor_add(
                            eg[:, :dce, :], eg[:, :dce, :],
                            xr_blk[:, :].unsqueeze(1).to_broadcast([P, dce, HID]))
                        nc.vector.scalar_tensor_tensor(
                            out=eg[:, :dce, :], in0=eg[:, :dce, :], scalar=NEG,
                            in1=eg[:, :dce, :], op0=AO.mult, op1=AO.max)
                        # score = <lrelu, att> per head
                        tmp = ep.tile([P, DCE, H, C], f32, tag="tmp")
                        nc.vector.tensor_tensor(
                            out=tmp[:, :dce, :, :],
                            in0=eg[:, :dce, :].rearrange("p d (h c) -> p d h c", h=H),
                            in1=ATTr[li][:, :].rearrange("p (h c) -> p h c", h=H)
                                .unsqueeze(1).to_broadcast([P, dce, H, C]),
                            op=AO.mult)
                        sc = ep.tile([P, DCE, H], f32, tag="sc")
                        nc.vector.tensor_reduce(
                            out=sc[:, :dce, :], in_=tmp[:, :dce, :, :],
                            axis=mybir.AxisListType.X, op=AO.add)
                        # mask fold + exp
                        nc.vector.scalar_tensor_tensor(
                            out=sc[:, :dce, :], in0=sc[:, :dce, :], scalar=SHIFT,
                            in1=maskc[:, mc:mc + dce].unsqueeze(2)
                                .to_broadcast([P, dce, H]),
                            op0=AO.add, op1=AO.mult)
                        ex = ep.tile([P, DCE, H], f32, tag="ex")
                        nc.scalar.activation(out=ex[:, :dce, :], in_=sc[:, :dce, :],
                                             func=AF.Exp, bias=negS[:, :1], scale=1.0)
                        # denominator partial
                        if j0 == 0:
                            nc.vector.tensor_reduce(
                                out=den[:], in_=ex[:, :dce, :].transpose([0, 2, 1]),
                                axis=mybir.AxisListType.X, op=AO.add)
                        else:
                            dpart = ep.tile([P, H], f32, tag="dpart")
                            nc.vector.tensor_reduce(
                                out=dpart[:], in_=ex[:, :dce, :].transpose([0, 2, 1]),
                                axis=mybir.AxisListType.X, op=AO.add)
                            nc.vector.tensor_add(den[:], den[:], dpart[:])
                        # weighted messages
                        nc.vector.tensor_tensor(
                            out=tmp[:, :dce, :, :],
                            in0=xlg[:, :dce, :].rearrange("p d (h c) -> p d h c", h=H),
                            in1=ex[:, :dce, :].unsqueeze(3).to_broadcast([P, dce, H, C]),
                            op=AO.mult)
                        if j0 == 0:
                            nc.vector.tensor_reduce(
                                out=wmsg[:].rearrange("p (h c) -> p h c", h=H),
                                in_=tmp[:, :dce, :, :].transpose([0, 2, 3, 1]),
                                axis=mybir.AxisListType.X, op=AO.add)
                        else:
                            mpart = ep.tile([P, HID], f32, tag="mpart")
                            nc.vector.tensor_reduce(
                                out=mpart[:].rearrange("p (h c) -> p h c", h=H),
                                in_=tmp[:, :dce, :, :].transpose([0, 2, 3, 1]),
                                axis=mybir.AxisListType.X, op=AO.add)
                            nc.vector.tensor_add(wmsg[:], wmsg[:], mpart[:])
                    # normalize + bias + relu -> next h_row
                    nc.vector.tensor_scalar_add(den[:], den[:], DEN_EPS)
                    rec = ap_.tile([P, H], f32, tag="rec")
                    nc.vector.reciprocal(rec[:], den[:])
                    nc.vector.tensor_tensor(
                        out=wmsg[:].rearrange("p (h c) -> p h c", h=H),
                        in0=wmsg[:].rearrange("p (h c) -> p h c", h=H),
                        in1=rec[:, :].unsqueeze(2).to_broadcast([P, H, C]),
                        op=AO.mult)
                    nc.vector.tensor_add(wmsg[:], wmsg[:], BIASr[li][:])
                    nc.scalar.activation(out=h_row[:, b, :], in_=wmsg[:],
                                         func=AF.Relu)

            # ---------------- pooling + head ----------------
            pool_ps = psS.tile([G, HID], f32, space="PSUM", tag="head")
            for b in range(NB):
                nc.tensor.matmul(pool_ps[:], lhsT=ind[:, b * G:(b + 1) * G],
                                 rhs=h_row[:, b, :],
                                 start=(b == 0), stop=(b == NB - 1))
            pl_sb = sp.tile([G, HID], f32, tag="plsb")
            nc.scalar.copy(pl_sb[:], pool_ps[:])
            nc.sync.dma_start(pool_in[:], pl_sb[:])
            nc.gpsimd.collective_compute(
                "AllReduce", AO.add, replica_groups=RG,
                ins=[pool_in.opt()], outs=[pool_out.opt()])
            gsum = sp.tile([G, HID], f32, tag="gsum")
            nc.sync.dma_start(gsum[:], pool_out[:])
            nc.vector.tensor_scalar_mul(gsum[:], gsum[:], invcnt[:, :1])

            def transpose_256(src, tag):
                gT = sp.tile([P, 2, G], f32, tag=tag)
                for g in range(2):
                    tp = psT.tile([P, P], f32, space="PSUM", tag="tps")
                    nc.tensor.transpose(out=tp[:, :G],
                                        in_=src[:, g * P:(g + 1) * P],
                                        identity=ident[:G, :G])
                    nc.scalar.copy(gT[:, g, :], tp[:, :G])
                return gT

            gT = transpose_256(gsum, "gT")
            mm1 = psS.tile([G, HID], f32, space="PSUM", tag="head")
            for g in range(2):
                nc.tensor.matmul(mm1[:], lhsT=gT[:, g, :], rhs=W1[:, g, :],
                                 start=(g == 0), stop=(g == 1))
            g1 = sp.tile([G, HID], f32, tag="g1")
            nc.vector.tensor_add(g1[:], mm1[:], b1_rep[:])
            nc.scalar.activation(out=g1[:], in_=g1[:], func=AF.Relu)
            g1T = transpose_256(g1, "g1T")
            mm2 = psS.tile([G, 8], f32, space="PSUM", tag="head")
            for g in range(2):
                nc.tensor.matmul(mm2[:], lhsT=g1T[:, g, :], rhs=W2[:, g, :],
                                 start=(g == 0), stop=(g == 1))
            ofin = sp.tile([G, 8], f32, tag="ofin")
            nc.vector.tensor_add(ofin[:], mm2[:], b2_rep[:])
            nc.sync.dma_start(out_d[:], ofin[:])

    nc.compile()
    return nc


# --------------------------------------------------------------------------
def run(cfg, inputs, trace=False, trace_cores=None):
    from concourse.bass_utils import run_bass_kernel_spmd
    Ds, S_off, core_inputs, shared = preprocess(cfg=cfg, **inputs)
    nc = build_program(cfg, Ds, S_off, cfg["NCORES"])
    in_maps = [{**shared, **ci} for ci in core_inputs]
    res = run_bass_kernel_spmd(nc, in_maps, core_ids=list(range(cfg["NCORES"])),
                               trace=trace, trace_cores=trace_cores)
    return res


def kernel(x, edge_index, edge_attr, batch, params):
    res = run(FULL_CFG, dict(x=x, edge_index=edge_index, edge_attr=edge_attr,
                             batch=batch, params=params))
    return np.asarray(res.results[0]["out"], np.float32)


# revision 14
# speedup vs baseline: 1.2429x; 1.2429x over previous
"""Trainium2 Bass kernel for the ASGRA GNN (3-layer GATv2 + pooling head).

Sharding: dst-node-range over 8 cores. Each core owns N/8 target nodes,
degree-sorts them, and processes its incoming edges in block-uniform
padded-degree grids of [128 nodes x D slots]. Per layer:
  - node transforms (lin_l/lin_r) computed data-parallel per 128-node block
    (PE transpose + PE matmul), xl AllGathered so every core can gather
    xl[src] for arbitrary sources (dma_gather, the only per-edge gather),
  - m = xl[src] + e_table[attr] + xr[dst] is accumulated on the TensorE:
    one-hot(attr) @ e_table plus identity matmuls of the gathered xl tile
    and the xr block (bf16 mode), so the edge phase needs no DVE adds and
    no e-table gather descriptors,
  - leaky-relu via DVE max(0.2x, x); attention scores via DVE mul+reduce;
    exp on ScalarE; segment softmax = free-dim reduction over the padded
    degree axis; padded slots masked via exp((s+30)*mask - 30),
  - weighted message sums accumulate unnormalized in f32, divided by the
    softmax denominator once per block.
Graph mean-pool = indicator matmul + AllReduce; MLP head replicated.
"""

import numpy as np
import ml_dtypes

P = 128
HID, H, C = 256, 4, 64
G = 64
NEG = 0.2
SHIFT = 30.0  # mask-fold shift: exp((s+SHIFT)*mask - SHIFT)
DEN_EPS = 1e-6
G_CH = 8      # slot columns per dma_gather
HALF = 4      # slot columns per PSUM m-tile / compute sub-chunk

FULL_CFG = dict(N=30000, E=480000, NCORES=8, NUM_TOKENS=151, NUM_REL=51,
                EMB=256, BBOX=32, IN0=288, XL_BF16=True)


def _derived(cfg):
    N, NCORES = cfg["N"], cfg["NCORES"]
    M = N // NCORES               # real nodes per core
    NB = (M + P - 1) // P         # node blocks per core
    MP = NB * P                   # padded nodes per core
    return M, NB, MP, NCORES * MP


def _wrap_idx(idx):
    """[S] int -> [128, S/16] int16 wrapped layout for dma_gather."""
    assert len(idx) % 16 == 0
    return np.ascontiguousarray(
        np.tile(idx.reshape(-1, 16).T, (8, 1)).astype(np.int16))


def preprocess(x, edge_index, edge_attr, batch, params, cfg):
    """Host-side sharding: per-core index/mask arrays + shared weights."""
    N, E, NCORES = cfg["N"], cfg["E"], cfg["NCORES"]
    M, NB, MP, NFULL = _derived(cfg)
    bf16 = ml_dtypes.bfloat16

    x = np.asarray(x, np.float32)
    ei = np.asarray(edge_index, np.int64)
    ea = np.asarray(edge_attr, np.int64)
    bt = np.asarray(batch, np.int64)
    src_g, dst_g = ei[0], ei[1]

    # ---- per-core degree-sorted node permutation + block schedule ----
    per_core = []
    deg_sorted_all = np.zeros((NCORES, MP), np.int64)
    for k in range(NCORES):
        lo = k * M
        emask = (dst_g >= lo) & (dst_g < lo + M)
        e_src = src_g[emask]
        e_dst = (dst_g[emask] - lo)
        e_att = ea[emask]
        deg = np.bincount(e_dst, minlength=M)
        perm = np.argsort(-deg, kind="stable")          # node order, degree desc
        deg_sorted_all[k, :M] = deg[perm]
        per_core.append(dict(lo=lo, perm=perm, e_src=e_src, e_dst=e_dst,
                             e_att=e_att, deg=deg))

    # shared block-degree schedule: max over cores so SPMD program is uniform
    Ds = [max(1, int(deg_sorted_all[:, b * P:(b + 1) * P].max()))
          for b in range(NB)]
    S_off = np.concatenate([[0], np.cumsum([P * D for D in Ds])]).astype(np.int64)
    S = int(S_off[-1])

    # ---- per-core slot arrays ----
    for k, cd in enumerate(per_core):
        perm, deg = cd["perm"], cd["deg"]
        starts = np.zeros(M + 1, np.int64)
        np.cumsum(deg, out=starts[1:])
        order = np.argsort(cd["e_dst"], kind="stable")
        gidx = np.zeros(S, np.int64)        # src gather index (permuted global)
        acol = np.zeros(S, np.float32)      # attr value per slot (one-hot input)
        mcol = np.zeros((P, S // P), np.float32)   # mask in [p, slot-col]

        for b in range(NB):
            base = S_off[b]
            for i in range(P):
                pos = b * P + i
                if pos >= M:
                    continue
                n = perm[pos]
                d = deg[n]
                eids = order[starts[n]:starts[n] + d]
                # slot (b, j, p) at flat index base + j*128 + p
                gidx[base + np.arange(d) * P + i] = cd["e_src"][eids]
                acol[base + np.arange(d) * P + i] = cd["e_att"][eids]
                mcol[i, base // P + np.arange(d)] = 1.0
        cd["gidx_raw"] = gidx
        cd["acol"] = acol
        cd["mcol"] = mcol

    # global -> permuted-global position map
    g2p = np.zeros(N, np.int64)
    for k, cd in enumerate(per_core):
        inv = np.argsort(cd["perm"])
        g2p[cd["lo"]:cd["lo"] + M] = k * MP + inv

    tok_full = x[:, 0].astype(np.int64)
    bbox_full = x[:, 1:5].astype(np.float32)

    cnt = np.bincount(bt, minlength=G).astype(np.float32)
    invcnt = (1.0 / np.maximum(cnt, 1.0)).astype(np.float32).reshape(G, 1)

    core_inputs = []
    for k, cd in enumerate(per_core):
        perm = cd["perm"]
        gidx = g2p[cd["gidx_raw"]]
        own_glob = np.zeros(MP, np.int64)
        own_glob[:M] = cd["lo"] + perm       # original ids in permuted order
        tok_idx = np.zeros(MP, np.int64)
        tok_idx[:M] = tok_full[own_glob[:M]]
        bbox_r = np.zeros((MP, 4), np.float32)
        bbox_r[:M] = bbox_full[own_glob[:M]]
        bbox_r = np.ascontiguousarray(
            bbox_r.reshape(-1, P, 4).transpose(1, 0, 2).reshape(P, -1))
        ind = np.zeros((P, NB * G), np.float32)
        for b in range(NB):
            for i in range(P):
                pos = b * P + i
                if pos < M:
                    ind[i, b * G + bt[own_glob[pos]]] = 1.0
        core_inputs.append(dict(
            gidx=_wrap_idx(gidx),
            acol=np.ascontiguousarray(cd["acol"].reshape(1, -1).astype(bf16)),
            mask=np.ascontiguousarray(cd["mcol"]),
            tok_idx=_wrap_idx(tok_idx),
            bbox_r=bbox_r, ind=ind, invcnt=invcnt))

    # ---- shared weights, preshaped ----
    p = params
    att_dt = bf16 if cfg.get("XL_BF16") else np.float32
    def A(v, dt=np.float32):
        return np.ascontiguousarray(np.asarray(v, np.float32).astype(dt))
    shared = dict(
        tok_emb=A(p["tok_emb"]),
        bbW=A(p["bbox_proj"]["W"]),
        bbb=A(p["bbox_proj"]["b"]).reshape(1, -1),
        relT=A(np.asarray(p["rel_emb"], np.float32).T),   # [C, NUM_REL]
        W1=A(p["mlp1"]["W"]), b1=A(p["mlp1"]["b"]).reshape(1, -1),
        W2=A(p["mlp2"]["W"]), b2=A(p["mlp2"]["b"]).reshape(1, -1),
        iota51=np.arange(cfg["NUM_REL"], dtype=np.float32).reshape(-1, 1),
    )
    for li, conv in enumerate(p["convs"]):
        shared[f"Wl{li}"] = A(conv["lin_l"]["W"])
        shared[f"Wr{li}"] = A(conv["lin_r"]["W"])
        shared[f"bl{li}"] = A(conv["lin_l"]["b"]).reshape(1, -1)
        shared[f"br{li}"] = A(conv["lin_r"]["b"]).reshape(1, -1)
        shared[f"bias{li}"] = A(conv["bias"]).reshape(1, -1)
        shared[f"att{li}"] = A(conv["att"], att_dt).reshape(1, -1)
        shared[f"leW{li}"] = A(conv["lin_e"]["W"])
    return Ds, S_off, core_inputs, shared


# --------------------------------------------------------------------------
def build_program(cfg, Ds, S_off, num_cores):
    import concourse.bacc as bacc
    import concourse.mybir as mybir
    import concourse.tile as tile
    from concourse.masks import make_identity

    NUM_TOKENS, NUM_REL = cfg["NUM_TOKENS"], cfg["NUM_REL"]
    EMB, BBOX, IN0 = cfg["EMB"], cfg["BBOX"], cfg["IN0"]
    XL_BF16 = bool(cfg.get("XL_BF16"))
    M, NB, MP, NFULL = _derived(cfg)
    S = int(S_off[-1])
    f32 = mybir.dt.float32
    bf = mybir.dt.bfloat16
    i16 = mybir.dt.int16
    dt_x = bf if XL_BF16 else f32      # xl table / xr / gathered tiles
    dt_m = bf if XL_BF16 else f32      # m / score-path tiles
    AO = mybir.AluOpType
    AF = mybir.ActivationFunctionType
    RG = [list(range(num_cores))]

    nc = bacc.Bacc("TRN2", target_bir_lowering=False, debug=False,
                   num_devices=num_cores)

    # ---------------- I/O ----------------
    din = {}
    def inp(name, shape, dt=f32):
        din[name] = nc.dram_tensor(name, list(shape), dt, kind="ExternalInput")
        return din[name]

    gidx_d = inp("gidx", (P, S // 16), i16)
    acol_d = inp("acol", (1, S), bf)
    mask_d = inp("mask", (P, S // P))
    tok_idx_d = inp("tok_idx", (P, MP // 16), i16)
    bbox_r_d = inp("bbox_r", (P, NB * 4))
    ind_d = inp("ind", (P, NB * G))
    invcnt_d = inp("invcnt", (G, 1))
    iota51_d = inp("iota51", (NUM_REL, 1))
    tok_emb_d = inp("tok_emb", (NUM_TOKENS, EMB))
    bbW_d = inp("bbW", (4, BBOX))
    bbb_d = inp("bbb", (1, BBOX))
    relT_d = inp("relT", (C, NUM_REL))
    W1_d = inp("W1", (HID, HID)); b1_d = inp("b1", (1, HID))
    W2_d = inp("W2", (HID, 8)); b2_d = inp("b2", (1, 8))
    Fs = [IN0, HID, HID]
    for li in range(3):
        inp(f"Wl{li}", (Fs[li], HID)); inp(f"Wr{li}", (Fs[li], HID))
        inp(f"bl{li}", (1, HID)); inp(f"br{li}", (1, HID))
        inp(f"bias{li}", (1, HID))
        inp(f"att{li}", (1, HID), dt_m)
        inp(f"leW{li}", (C, HID))
    out_d = nc.dram_tensor("out", [G, 8], f32, kind="ExternalOutput")

    with tile.TileContext(nc) as tc:
        with (
            nc.allow_low_precision("mixed-precision GNN kernel"),
            tc.tile_pool(name="const", bufs=1) as cp,
            tc.tile_pool(name="stream", bufs=3) as sp,
            tc.tile_pool(name="edge", bufs=3) as ep,
            tc.tile_pool(name="acc", bufs=2) as ap_,
            tc.tile_pool(name="psT", bufs=2, space="PSUM") as psT,
            tc.tile_pool(name="psM", bufs=2, space="PSUM") as psM,
            tc.tile_pool(name="psE", bufs=4, space="PSUM") as psE,
            tc.tile_pool(name="dram", bufs=1, space="DRAM") as dp,
        ):
            ident = cp.tile([P, P], f32)
            make_identity(nc, ident[:])
            negS = cp.tile([P, 1], f32)
            nc.gpsimd.memset(negS[:], -SHIFT)
            if XL_BF16:
                ident_b = cp.tile([P, P], bf)
                nc.scalar.copy(ident_b[:], ident[:])

            # persistent SBUF loads
            gidx = cp.tile([P, S // 16], i16)
            nc.sync.dma_start(gidx[:], gidx_d[:])
            maskc = cp.tile([P, S // P], f32)
            nc.sync.dma_start(maskc[:], mask_d[:])
            tok_idx = cp.tile([P, MP // 16], i16)
            nc.sync.dma_start(tok_idx[:], tok_idx_d[:])
            ind = cp.tile([P, NB * G], f32)
            nc.sync.dma_start(ind[:], ind_d[:])
            invcnt = cp.tile([G, 1], f32)
            nc.sync.dma_start(invcnt[:], invcnt_d[:])
            iota51 = cp.tile([NUM_REL, 1], f32)
            nc.sync.dma_start(iota51[:], iota51_d[:])
            bbox_r = cp.tile([P, NB * 4], f32)
            nc.sync.dma_start(bbox_r[:], bbox_r_d[:])
            bbW = cp.tile([4, BBOX], f32)
            nc.sync.dma_start(bbW[:], bbW_d[:])
            bbb_rep = cp.tile([P, BBOX], f32)
            nc.sync.dma_start(bbb_rep[:], bbb_d[:].to_broadcast([P, BBOX]))
            relT = cp.tile([C, NUM_REL], f32)
            nc.sync.dma_start(relT[:], relT_d[:])

            WL, WR, BLr, BRr, BIASr, ATTr, LEW = [], [], [], [], [], [], []
            for li in range(3):
                F = Fs[li]
                KG = (F + P - 1) // P
                wl = cp.tile([P, KG, HID], f32, name=f"wl{li}", tag=f"wl{li}")
                wr = cp.tile([P, KG, HID], f32, name=f"wr{li}", tag=f"wr{li}")
                for g in range(KG):
                    ksz = min(P, F - g * P)
                    nc.sync.dma_start(wl[:ksz, g, :], din[f"Wl{li}"][g * P:g * P + ksz, :])
                    nc.sync.dma_start(wr[:ksz, g, :], din[f"Wr{li}"][g * P:g * P + ksz, :])
                WL.append(wl); WR.append(wr)
                for lst, nm in ((BLr, "bl"), (BRr, "br"), (BIASr, "bias")):
                    t = cp.tile([P, HID], f32, name=f"{nm}{li}r", tag=f"{nm}{li}r")
                    nc.sync.dma_start(t[:], din[f"{nm}{li}"][:].to_broadcast([P, HID]))
                    lst.append(t)
                ta = cp.tile([P, HID], dt_m, name=f"att{li}r", tag=f"att{li}r")
                nc.sync.dma_start(ta[:], din[f"att{li}"][:].to_broadcast([P, HID]))
                ATTr.append(ta)
                lew = cp.tile([C, HID], f32, name=f"lew{li}", tag=f"lew{li}")
                nc.sync.dma_start(lew[:], din[f"leW{li}"][:])
                LEW.append(lew)
            ET = []
            for li in range(3):
                et = cp.tile([NUM_REL, HID], bf, name=f"etab{li}", tag=f"etab{li}")
                ET.append(et)

            W1 = cp.tile([P, 2, HID], f32)
            W2 = cp.tile([P, 2, 8], f32)
            for g in range(2):
                nc.sync.dma_start(W1[:, g, :], W1_d[g * P:(g + 1) * P, :])
                nc.sync.dma_start(W2[:, g, :], W2_d[g * P:(g + 1) * P, :])
            b1_rep = cp.tile([G, HID], f32)
            nc.sync.dma_start(b1_rep[:], b1_d[:].to_broadcast([G, HID]))
            b2_rep = cp.tile([G, 8], f32)
            nc.sync.dma_start(b2_rep[:], b2_d[:].to_broadcast([G, 8]))

            # node features of current layer, row-major [p, block, ch]
            h_row = cp.tile([P, NB, HID], f32)

            # per-layer DRAM buffers
            xl_own = [dp.tile([MP, HID], dt_x, name=f"xl_own{i}") for i in range(3)]
            xl_full = [dp.tile([NFULL, HID], dt_x, name=f"xl_full{i}") for i in range(3)]
            xr_own = [dp.tile([MP, HID], dt_x, name=f"xr_own{i}") for i in range(3)]
            pool_in = dp.tile([G, HID], f32)
            pool_out = dp.tile([G, HID], f32)

            for li in range(3):
                F = Fs[li]
                KG = (F + P - 1) // P

                # ---- e_table = rel_emb @ lin_e.W  (kept in SBUF, bf16) ----
                et_ps = psM.tile([NUM_REL, HID], f32, space="PSUM", tag="mmps")
                nc.tensor.matmul(et_ps[:], lhsT=relT[:], rhs=LEW[li][:],
                                 start=True, stop=True)
                nc.scalar.copy(ET[li][:], et_ps[:])

                # ---- node transforms, streamed per block ----
                for b in range(NB):
                    if li == 0:
                        # layer-0 input built on the fly: tok emb gather + bbox
                        tok_blk = sp.tile([P, 1, EMB], f32, tag="tokb")
                        nc.gpsimd.dma_gather(
                            tok_blk[:], tok_emb_d[:],
                            tok_idx[:, b * 8:(b + 1) * 8], P, P, EMB)
                        bbT_ps = psT.tile([P, P], f32, space="PSUM", tag="tps")
                        nc.tensor.transpose(out=bbT_ps[:4, :],
                                            in_=bbox_r[:, b * 4:(b + 1) * 4],
                                            identity=ident[:])
                        bbT = sp.tile([4, P], f32, tag="bbT")
                        nc.scalar.copy(bbT[:], bbT_ps[:4, :])
                        bb_ps = psM.tile([P, BBOX], f32, space="PSUM", tag="mmps")
                        nc.tensor.matmul(bb_ps[:], lhsT=bbT[:],
                                         rhs=bbW[:], start=True, stop=True)
                        bb_sb = sp.tile([P, BBOX], f32, tag="bbsb")
                        nc.vector.tensor_add(bb_sb[:], bb_ps[:], bbb_rep[:])
                    hT = sp.tile([P, KG, P], f32, tag="hT")
                    for g in range(KG):
                        ksz = min(P, F - g * P)
                        if li == 0:
                            src_ap = (tok_blk[:, 0, g * P:g * P + ksz] if g < 2
                                      else bb_sb[:])
                        else:
                            src_ap = h_row[:, b, g * P:g * P + ksz]
                        tp = psT.tile([P, P], f32, space="PSUM", tag="tps")
                        nc.tensor.transpose(out=tp[:ksz, :], in_=src_ap,
                                            identity=ident[:])
                        nc.scalar.copy(hT[:ksz, g, :], tp[:ksz, :])
                    for dst_dram, W, brep in ((xl_own[li], WL[li], BLr[li]),
                                              (xr_own[li], WR[li], BRr[li])):
                        mm = psM.tile([P, HID], f32, space="PSUM", tag="mmps")
                        for g in range(KG):
                            ksz = min(P, F - g * P)
                            nc.tensor.matmul(mm[:], lhsT=hT[:ksz, g, :],
                                             rhs=W[:ksz, g, :],
                                             start=(g == 0), stop=(g == KG - 1))
                        xsb = sp.tile([P, HID], dt_x, tag="xsb")
                        nc.vector.tensor_add(xsb[:], mm[:], brep[:])
                        nc.sync.dma_start(dst_dram[b * P:(b + 1) * P, :], xsb[:])

                # ---- AllGather xl ----
                nc.gpsimd.collective_compute(
                    "AllGather", AO.bypass, replica_groups=RG,
                    ins=[xl_own[li].opt()], outs=[xl_full[li].opt()])

                # ---- edge phase ----
                for b in range(NB):
                    D = Ds[b]
                    base = int(S_off[b])
                    xr_blk = sp.tile([P, HID], dt_x, tag="xrb")
                    nc.sync.dma_start(xr_blk[:], xr_own[li][b * P:(b + 1) * P, :])
                    wmsg = ap_.tile([P, HID], f32, tag="wmsg")
                    den = ap_.tile([P, H], f32, tag="den")
                    first = True
                    for g0 in range(0, D, G_CH):
                        cg = min(G_CH, D - g0)
                        ni = cg * P
                        sl0 = base + g0 * P
                        xlg = ep.tile([P, G_CH, HID], dt_x, tag="xlg")
                        nc.gpsimd.dma_gather(
                            xlg[:, :cg, :], xl_full[li][:],
                            gidx[:, sl0 // 16:sl0 // 16 + ni // 16], ni, ni, HID)
                        attr_rep = ep.tile([NUM_REL, G_CH * P], bf, tag="attr")
                        nc.sync.dma_start(
                            attr_rep[:, :ni],
                            acol_d[0:1, sl0:sl0 + ni].to_broadcast([NUM_REL, ni]))
                        onehot = ep.tile([NUM_REL, G_CH * P], bf, tag="onehot")
                        nc.vector.tensor_scalar(
                            out=onehot[:, :ni], in0=attr_rep[:, :ni],
                            scalar1=iota51[:, :1], scalar2=None, op0=AO.is_equal)
                        for h0 in range(0, cg, HALF):
                            hc = min(HALF, cg - h0)
                            mc = base // P + g0 + h0
                            m_sb = ep.tile([P, HALF, HID], dt_m, tag="msb")
                            cols = []
                            for j in range(hc):
                                pcol = psE.tile([P, HID], f32, space="PSUM",
                                                tag="psm", name=f"psm{j}")
                                cols.append(pcol)
                                if XL_BF16:
                                    # m = xl + e[attr] + xr accumulated on PE
                                    nc.tensor.matmul(
                                        pcol[:], lhsT=ident_b[:],
                                        rhs=xlg[:, h0 + j, :],
                                        start=True, stop=False,
                                        skip_group_check=True)
                                    nc.tensor.matmul(
                                        pcol[:],
                                        lhsT=onehot[:, (h0 + j) * P:(h0 + j + 1) * P],
                                        rhs=ET[li][:],
                                        start=False, stop=False,
                                        skip_group_check=True)
                                    nc.tensor.matmul(
                                        pcol[:], lhsT=ident_b[:],
                                        rhs=xr_blk[:],
                                        start=False, stop=True,
                                        skip_group_check=True)
                                    nc.scalar.copy(m_sb[:, j, :], pcol[:])
                                else:
                                    nc.tensor.matmul(
                                        pcol[:],
                                        lhsT=onehot[:, (h0 + j) * P:(h0 + j + 1) * P],
                                        rhs=ET[li][:],
                                        start=True, stop=True,
                                        skip_group_check=True)
                            if XL_BF16:
                                nc.vector.scalar_tensor_tensor(
                                    out=m_sb[:, :hc, :], in0=m_sb[:, :hc, :],
                                    scalar=NEG, in1=m_sb[:, :hc, :],
                                    op0=AO.mult, op1=AO.max)
                            else:
                                for j in range(hc):
                                    nc.vector.tensor_add(
                                        m_sb[:, j, :], xlg[:, h0 + j, :],
                                        cols[j][:])
                                nc.vector.tensor_add(
                                    m_sb[:, :hc, :], m_sb[:, :hc, :],
                                    xr_blk[:, :].unsqueeze(1)
                                        .to_broadcast([P, hc, HID]))
                                nc.vector.scalar_tensor_tensor(
                                    out=m_sb[:, :hc, :], in0=m_sb[:, :hc, :],
                                    scalar=NEG, in1=m_sb[:, :hc, :],
                                    op0=AO.mult, op1=AO.max)
                            # score = <lrelu(m), att> per head
                            tmp = ep.tile([P, HALF, H, C], dt_m, tag="tmp")
                            nc.vector.tensor_tensor(
                                out=tmp[:, :hc, :, :],
                                in0=m_sb[:, :hc, :]
                                    .rearrange("p d (h c) -> p d h c", h=H),
                                in1=ATTr[li][:, :].rearrange("p (h c) -> p h c", h=H)
                                    .unsqueeze(1).to_broadcast([P, hc, H, C]),
                                op=AO.mult)
                            sc = ep.tile([P, HALF, H], f32, tag="sc")
                            nc.vector.tensor_reduce(
                                out=sc[:, :hc, :], in_=tmp[:, :hc, :, :],
                                axis=mybir.AxisListType.X, op=AO.add)
                            nc.vector.scalar_tensor_tensor(
                                out=sc[:, :hc, :], in0=sc[:, :hc, :], scalar=SHIFT,
                                in1=maskc[:, mc:mc + hc].unsqueeze(2)
                                    .to_broadcast([P, hc, H]),
                                op0=AO.add, op1=AO.mult)
                            ex = ep.tile([P, HALF, H], dt_m, tag="ex")
                            nc.scalar.activation(
                                out=ex[:, :hc, :], in_=sc[:, :hc, :],
                                func=AF.Exp, bias=negS[:, :1], scale=1.0)
                            if first:
                                nc.vector.tensor_reduce(
                                    out=den[:],
                                    in_=ex[:, :hc, :].transpose([0, 2, 1]),
                                    axis=mybir.AxisListType.X, op=AO.add)
                            else:
                                dpart = ep.tile([P, H], f32, tag="dpart")
                                nc.vector.tensor_reduce(
                                    out=dpart[:],
                                    in_=ex[:, :hc, :].transpose([0, 2, 1]),
                                    axis=mybir.AxisListType.X, op=AO.add)
                                nc.vector.tensor_add(den[:], den[:], dpart[:])
                            # weighted messages
                            nc.vector.tensor_tensor(
                                out=tmp[:, :hc, :, :],
                                in0=xlg[:, h0:h0 + hc, :]
                                    .rearrange("p d (h c) -> p d h c", h=H),
                                in1=ex[:, :hc, :].unsqueeze(3)
                                    .to_broadcast([P, hc, H, C]),
                                op=AO.mult)
                            if first:
                                nc.vector.tensor_reduce(
                                    out=wmsg[:].rearrange("p (h c) -> p h c", h=H),
                                    in_=tmp[:, :hc, :, :].transpose([0, 2, 3, 1]),
                                    axis=mybir.AxisListType.X, op=AO.add)
                            else:
                                mpart = ep.tile([P, HID], f32, tag="mpart")
                                nc.vector.tensor_reduce(
                                    out=mpart[:].rearrange("p (h c) -> p h c", h=H),
                                    in_=tmp[:, :hc, :, :].transpose([0, 2, 3, 1]),
                                    axis=mybir.AxisListType.X, op=AO.add)
                                nc.vector.tensor_add(wmsg[:], wmsg[:], mpart[:])
                            first = False
                    # normalize + bias + relu -> next h_row
                    nc.vector.tensor_scalar_add(den[:], den[:], DEN_EPS)
                    rec = ap_.tile([P, H], f32, tag="rec")
                    nc.vector.reciprocal(rec[:], den[:])
                    nc.vector.tensor_tensor(
                        out=wmsg[:].rearrange("p (h c) -> p h c", h=H),
                        in0=wmsg[:].rearrange("p (h c) -> p h c", h=H),
                        in1=rec[:, :].unsqueeze(2).to_broadcast([P, H, C]),
                        op=AO.mult)
                    nc.vector.tensor_add(wmsg[:], wmsg[:], BIASr[li][:])
                    nc.scalar.activation(out=h_row[:, b, :], in_=wmsg[:],
                                         func=AF.Relu)

            # ---------------- pooling + head ----------------
            pool_ps = psM.tile([G, HID], f32, space="PSUM", tag="mmps")
            for b in range(NB):
                nc.tensor.matmul(pool_ps[:], lhsT=ind[:, b * G:(b + 1) * G],
                                 rhs=h_row[:, b, :],
                                 start=(b == 0), stop=(b == NB - 1))
            pl_sb = sp.tile([G, HID], f32, tag="plsb")
            nc.scalar.copy(pl_sb[:], pool_ps[:])
            nc.sync.dma_start(pool_in[:], pl_sb[:])
            nc.gpsimd.collective_compute(
                "AllReduce", AO.add, replica_groups=RG,
                ins=[pool_in.opt()], outs=[pool_out.opt()])
            gsum = sp.tile([G, HID], f32, tag="gsum")
            nc.sync.dma_start(gsum[:], pool_out[:])
            nc.vector.tensor_scalar_mul(gsum[:], gsum[:], invcnt[:, :1])

            def transpose_256(src, tag):
                gT = sp.tile([P, 2, G], f32, tag=tag)
                for g in range(2):
                    tp = psT.tile([P, P], f32, space="PSUM", tag="tps")
                    nc.tensor.transpose(out=tp[:, :G],
                                        in_=src[:, g * P:(g + 1) * P],
                                        identity=ident[:G, :G])
                    nc.scalar.copy(gT[:, g, :], tp[:, :G])
                return gT

            gT = transpose_256(gsum, "gT")
            mm1 = psM.tile([G, HID], f32, space="PSUM", tag="mmps")
            for g in range(2):
                nc.tensor.matmul(mm1[:], lhsT=gT[:, g, :], rhs=W1[:, g, :],
                                 start=(g == 0), stop=(g == 1))
            g1 = sp.tile([G, HID], f32, tag="g1")
            nc.vector.tensor_add(g1[:], mm1[:], b1_rep[:])
            nc.scalar.activation(out=g1[:], in_=g1[:], func=AF.Relu)
            g1T = transpose_256(g1, "g1T")
            mm2 = psM.tile([G, 8], f32, space="PSUM", tag="mmps")
            for g in range(2):
                nc.tensor.matmul(mm2[:], lhsT=g1T[:, g, :], rhs=W2[:, g, :],
                                 start=(g == 0), stop=(g == 1))
            ofin = sp.tile([G, 8], f32, tag="ofin")
            nc.vector.tensor_add(ofin[:], mm2[:], b2_rep[:])
            nc.sync.dma_start(out_d[:], ofin[:])

    nc.compile()
    return nc


# --------------------------------------------------------------------------
def run(cfg, inputs, trace=False, trace_cores=None):
    from concourse.bass_utils import run_bass_kernel_spmd
    Ds, S_off, core_inputs, shared = preprocess(cfg=cfg, **inputs)
    nc = build_program(cfg, Ds, S_off, cfg["NCORES"])
    in_maps = [{**shared, **ci} for ci in core_inputs]
    res = run_bass_kernel_spmd(nc, in_maps, core_ids=list(range(cfg["NCORES"])),
                               trace=trace, trace_cores=trace_cores)
    return res


def kernel(x, edge_index, edge_attr, batch, params):
    res = run(FULL_CFG, dict(x=x, edge_index=edge_index, edge_attr=edge_attr,
                             batch=batch, params=params))
    return np.asarray(res.results[0]["out"], np.float32)
